# revision 8
# baseline (speedup 1.0000x reference)
"""Trainium2 Bass kernel for nn_HSIKeyBankAttention.

Strategy (8 NeuronCores, data-parallel over batch B=8, one batch row per core):

Per core, x_b is [N=4096, C=512]. The whole computation is restructured around
two SBUF layouts:
  * feature-major  [feature-on-partitions, N free]  (xT, qT)
  * token-major    [tokens-on-partitions,  feature free]  (v, t, logits, sims, E, out)

Host-side (cheap, tiny tensors / layout only):
  * x is cast to fp16 and pre-transposed per core (xT), weights are fused:
      - Wt      = Wv @ blockdiag_h(W_tok)              (t_raw = v_h @ W_tok)
      - Ws_spec = Wv @ blockdiag_h(W_tok @ pn_spec^T)  (s_raw = t_raw . pn)
      - bank blockdiag B[h*16+m, h*64+d] = K_spec[h,m,d]
  * prototype normalization pn = l2norm(K @ W_prot + b_prot) (8*16*64 elements)
  * the diversity loss (pure function of K_spec/K_spat) is computed on host.

Device pipeline per core:
  S2  big pass:   [v | t | l_spec l_spat s_spec s_spat] = x @ Wbig   (n-major)
                  evac: v->fp16, t->sum(t^2) per head, exp(l)->y, s->fp16
  S1  qT pass:    qT = (Wq^T x^T)  (feature-major, weights stationary)
  S3  top-3 masked renormalized softmax over y (16-wide segments) -> r
  S4  r -> rT (DMA transpose), K_eff = r @ B (psum), E = exp(K_eff)
  S5  ctx_h^T = v_h^T E_h (psum accum over 32 chunks) + Z = ones^T E
  S6  W_eff = blockdiag(ctx^T) @ W_proj, row-scaled by 1/Z
  S7  out = q @ W_eff  (n-major), DMA out
  S8  loss reductions: pos = sum_m s.r, lse = ln sum_m exp(s*rsn/tau)

Host gathers the 8 per-core outputs and combines the loss partial sums.
"""

import sys

if "/opt/trn_rl_repo" not in sys.path:
    sys.path.insert(0, "/opt/trn_rl_repo")

from contextlib import ExitStack

import numpy as np

import concourse.bass as bass
import concourse.bacc as bacc
import concourse.mybir as mybir
import concourse.tile as tile
from concourse.bass_utils import run_bass_kernel_spmd

F16 = mybir.dt.float16
F32 = mybir.dt.float32
AX = mybir.AxisListType
OP = mybir.AluOpType
AF = mybir.ActivationFunctionType

B, N, C, H, D, M = 8, 4096, 512, 8, 64, 16
TOPK = 3
TAU = 0.1
DIV_REG = 0.01
NCORES = 8


def build_core_kernel(NB=32, with_big_bias=False, with_proj_bias=False):
    """Build the per-core Bass/Tile kernel. NB = number of 128-token chunks."""
    Np = NB * 128
    nc = bacc.Bacc("TRN2", target_bir_lowering=False, debug=False, num_devices=NCORES)

    qseg = min(512, Np)
    NSEG = Np // qseg
    GSZ = qseg // 128
    xT_d = nc.dram_tensor("xT", [128, NSEG, 4, qseg], F16, kind="ExternalInput")
    wq_d = nc.dram_tensor("wq", [128, 4, 512], F16, kind="ExternalInput")
    wbig_d = nc.dram_tensor("wbig", [128, 4, 1536], F16, kind="ExternalInput")
    bcat_d = nc.dram_tensor("bcat", [128, 1024], F16, kind="ExternalInput")
    wproj_d = nc.dram_tensor("wproj", [128, 4, 512], F16, kind="ExternalInput")
    brow_d = nc.dram_tensor("brow", [1, 2048], F16, kind="ExternalInput")
    out_d = nc.dram_tensor("out", [Np, 512], F32, kind="ExternalOutput")
    loss_d = nc.dram_tensor("lossp", [1, 2], F32, kind="ExternalOutput")

    with tile.TileContext(nc) as tc, ExitStack() as ctx:
        const = ctx.enter_context(tc.tile_pool(name="const", bufs=1))

        wq = const.tile([128, 4, 512], F16)
        nc.sync.dma_start(out=wq[:], in_=wq_d[:])
        wbig = const.tile([128, 4, 1536], F16)
        nc.sync.dma_start(out=wbig[:], in_=wbig_d[:])
        bcat = const.tile([128, 1024], F16)
        nc.sync.dma_start(out=bcat[:], in_=bcat_d[:])
        wproj = const.tile([128, 4, 512], F16)
        nc.sync.dma_start(out=wproj[:], in_=wproj_d[:])
        brow = const.tile([1, 2048], F16)
        nc.sync.dma_start(out=brow[:], in_=brow_d[:])

        ones16 = const.tile([128, 1], F16)
        nc.any.memset(ones16[:], 1.0)
        ones32 = const.tile([128, 1], F32)
        nc.any.memset(ones32[:], 1.0)
        onesrow = const.tile([1, 128], F16)
        nc.any.memset(onesrow[:], 1.0)

        # persistent intermediates
        vsb = const.tile([128, NB, 512], F16)       # v  (n-major)
        ysb = const.tile([128, NB, 256], F16)       # exp(logits), both banks
        ssb = const.tile([128, NB, 256], F16)       # raw sims, both banks
        nrm2 = const.tile([128, NB, 8], F32)        # ||t_raw||^2 per (n, h)
        qT = const.tile([128, 4, Np], F16)          # q  (feature-major)

        # ------- S2+S1: big x pass + qT pass, streaming xT by 512-col group
        with tc.tile_pool(name="xgp", bufs=2) as xgp, \
             tc.tile_pool(name="ps_big", bufs=2, space="PSUM") as psb, \
             tc.tile_pool(name="ps_q", bufs=2, space="PSUM") as psq, \
             tc.tile_pool(name="sqp", bufs=3) as sqp:
            for ns in range(NSEG):
                xg = xgp.tile([128, 4, qseg], F16, tag="xg")
                nc.sync.dma_start(out=xg[:], in_=xT_d[:, ns])
                for nbi in range(GSZ):
                    nb = ns * GSZ + nbi
                    pv = psb.tile([128, 512], F32, tag="pv")
                    pt = psb.tile([128, 512], F32, tag="pt")
                    pr = psb.tile([128, 512], F32, tag="pr")
                    if with_big_bias:
                        nc.tensor.matmul(pv[:], onesrow[:1, :], brow[:1, 0:512],
                                         start=True, stop=False)
                        nc.tensor.matmul(pt[:], onesrow[:1, :],
                                         brow[:1, 512:1024],
                                         start=True, stop=False)
                        nc.tensor.matmul(pr[:], onesrow[:1, :],
                                         brow[:1, 1024:1536],
                                         start=True, stop=False)
                    for cb in range(4):
                        lhs = xg[:, cb, nbi * 128:(nbi + 1) * 128]
                        st = (cb == 0) and not with_big_bias
                        sp = (cb == 3)
                        nc.tensor.matmul(pv[:], lhs, wbig[:, cb, 0:512],
                                         start=st, stop=sp)
                        nc.tensor.matmul(pt[:], lhs, wbig[:, cb, 512:1024],
                                         start=st, stop=sp)
                        nc.tensor.matmul(pr[:], lhs, wbig[:, cb, 1024:1536],
                                         start=st, stop=sp)
                    nc.vector.tensor_copy(vsb[:, nb], pv[:])
                    sq = sqp.tile([128, 512], F32, tag="sq")
                    nc.scalar.square(sq[:], pt[:])
                    nc.vector.tensor_reduce(
                        nrm2[:, nb], sq[:].rearrange("p (h d) -> p h d", d=64),
                        axis=AX.X, op=OP.add)
                    nc.scalar.activation(ysb[:, nb], pr[:, 0:256], AF.Exp)
                    nc.vector.tensor_copy(ssb[:, nb], pr[:, 256:512])
                for j in range(4):
                    pq = psq.tile([128, qseg], F32, tag="pq")
                    for cb in range(4):
                        nc.tensor.matmul(
                            pq[:], wq[:, cb, j * 128:(j + 1) * 128],
                            xg[:, cb, :],
                            start=(cb == 0), stop=(cb == 3))
                    nc.scalar.copy(qT[:, j, ns * qseg:(ns + 1) * qseg], pq[:])

        # ------- S3/S4/S5 in halves: topk -> rT -> K_eff/E -> ctx/Z --------
        NH = 2 if (NB >= 8 and NB % 2 == 0) else 1
        HB = NB // NH
        rsb = const.tile([128, 2, NB, 128], F16)    # r (n-major)
        ctxp = ctx.enter_context(tc.tile_pool(name="ps_ctx", bufs=1, space="PSUM"))
        pz = ctxp.tile([1, 512], F32, tag="pz")
        pcs = [ctxp.tile([128, 128], F32, tag=f"pc{i}", name=f"pc{i}")
               for i in range(4)]
        with tc.tile_pool(name="tkp", bufs=1) as tkp, \
             tc.tile_pool(name="rtp", bufs=3) as rtp, \
             tc.tile_pool(name="erot", bufs=3) as erot, \
             tc.tile_pool(name="ps_ke", bufs=2, space="PSUM") as kep:
            for h0 in range(NH):
                hsl = slice(h0 * HB, (h0 + 1) * HB)
                for bank in range(2):
                    y4 = ysb[:, hsl, bank * 128:(bank + 1) * 128].rearrange(
                        "p a (h m) -> p a h m", m=16)
                    t1 = tkp.tile([128, HB * 8], F16, tag="t1")
                    t2 = tkp.tile([128, HB * 8], F16, tag="t2")
                    t3 = tkp.tile([128, HB * 8], F16, tag="t3")
                    d = tkp.tile([128, HB, 128], F16, tag="d")
                    w1 = tkp.tile([128, HB, 128], F16, tag="w1")
                    w2 = tkp.tile([128, HB, 128], F16, tag="w2")
                    d4 = d[:].rearrange("p a (h m) -> p a h m", m=16)
                    w14 = w1[:].rearrange("p a (h m) -> p a h m", m=16)
                    w24 = w2[:].rearrange("p a (h m) -> p a h m", m=16)

                    def bc(t):
                        return t[:].rearrange(
                            "p (a h) -> p a h", h=8).unsqueeze(
                            3).broadcast_to([128, HB, 8, 16])

                    nc.vector.tensor_reduce(t1[:], y4, axis=AX.X, op=OP.max)
                    nc.vector.tensor_tensor(d4, y4, bc(t1), op=OP.subtract)
                    nc.vector.scalar_tensor_tensor(
                        w14, d4, 0.0, y4, op0=OP.is_lt, op1=OP.mult)
                    nc.vector.tensor_reduce(t2[:], w14, axis=AX.X, op=OP.max)
                    nc.vector.tensor_tensor(d4, w14, bc(t2), op=OP.subtract)
                    nc.vector.scalar_tensor_tensor(
                        w24, d4, 0.0, w14, op0=OP.is_lt, op1=OP.mult)
                    nc.vector.tensor_reduce(t3[:], w24, axis=AX.X, op=OP.max)
                    nc.vector.tensor_tensor(d4, y4, bc(t3), op=OP.subtract)
                    nc.vector.scalar_tensor_tensor(
                        w14, d4, 0.0, y4, op0=OP.is_ge, op1=OP.mult)  # ym
                    s3a = tkp.tile([128, HB * 8], F32, tag="s3a")
                    s3b = tkp.tile([128, HB * 8], F32, tag="s3b")
                    rs32 = tkp.tile([128, HB * 8], F32, tag="rs32")
                    rs16 = tkp.tile([128, HB * 8], F16, tag="rs16")
                    nc.vector.tensor_tensor(s3a[:], t1[:], t2[:], op=OP.add)
                    nc.vector.tensor_tensor(s3b[:], s3a[:], t3[:], op=OP.add)
                    nc.vector.reciprocal(rs32[:], s3b[:])
                    nc.vector.tensor_copy(rs16[:], rs32[:])
                    r4 = rsb[:, bank, hsl].rearrange(
                        "p a (h m) -> p a h m", m=16)
                    rs16b = rs16[:].rearrange(
                        "p (a h) -> p a h", h=8).unsqueeze(
                        3).broadcast_to([128, HB, 8, 16])
                    nc.vector.tensor_tensor(r4, w14, rs16b, op=OP.mult)
                for nb in range(h0 * HB, (h0 + 1) * HB):
                    rtt = rtp.tile([128, 2, 128], F16, tag="rtt")
                    nc.sync.dma_start(out=rtt[:, 0], in_=rsb[:, 0, nb],
                                      transpose=True)
                    nc.sync.dma_start(out=rtt[:, 1], in_=rsb[:, 1, nb],
                                      transpose=True)
                    pk = kep.tile([128, 512], F32, tag="pk")
                    nc.tensor.matmul(pk[:], rtt[:, 0], bcat[:, 0:512],
                                     start=True, stop=False)
                    nc.tensor.matmul(pk[:], rtt[:, 1], bcat[:, 512:1024],
                                     start=False, stop=True)
                    Et = erot.tile([128, 512], F16, tag="Et")
                    nc.scalar.activation(Et[:], pk[:], AF.Exp)
                    for p in range(4):
                        nc.tensor.matmul(
                            pcs[p][:], vsb[:, nb, p * 128:(p + 1) * 128],
                            Et[:, p * 128:(p + 1) * 128],
                            start=(nb == 0), stop=(nb == NB - 1))
                    nc.tensor.matmul(pz[:], ones16[:, 0:1], Et[:],
                                     start=(nb == 0), stop=(nb == NB - 1))

        # ---------------- S6: Z recip + W_eff ------------------------------
        small = ctx.enter_context(tc.tile_pool(name="small", bufs=1))
        zr = small.tile([1, 512], F32)
        nc.vector.reciprocal(zr[:], pz[:])
        zrT = small.tile([128, 4], F32)
        for j in range(4):
            nc.sync.dma_start(out=zrT[:, j:j + 1],
                              in_=zr[0:1, j * 128:(j + 1) * 128])
        bd = small.tile([128, 4, 128], F16)
        nc.any.memset(bd[:], 0.0)
        for p in range(4):
            for h2 in range(2):
                sl = slice(h2 * 64, (h2 + 1) * 64)
                nc.scalar.copy(bd[sl, p, sl], pcs[p][sl, sl])
        weff = small.tile([128, 4, 512], F16)
        with tc.tile_pool(name="ps_w", bufs=2, space="PSUM") as pwp:
            for j in range(4):
                pw = pwp.tile([128, 512], F32, tag="pw")
                nc.tensor.matmul(pw[:], bd[:, j], wproj[:, j],
                                 start=True, stop=True)
                nc.scalar.activation(weff[:, j], pw[:], AF.Copy,
                                     scale=zrT[:, j:j + 1])

        # ---------------- S7: out = q @ W_eff ------------------------------
        with tc.tile_pool(name="ps_o", bufs=2, space="PSUM") as pso, \
             tc.tile_pool(name="outst", bufs=3) as outst:
            for nb in range(NB):
                po = pso.tile([128, 512], F32, tag="po")
                if with_proj_bias:
                    nc.tensor.matmul(po[:], onesrow[:1, :], brow[:1, 1536:2048],
                                     start=True, stop=False)
                for j in range(4):
                    nc.tensor.matmul(
                        po[:], qT[:, j, nb * 128:(nb + 1) * 128], weff[:, j],
                        start=(j == 0 and not with_proj_bias), stop=(j == 3))
                ost = outst.tile([128, 512], F32, tag="ost")
                nc.scalar.copy(ost[:], po[:])
                nc.sync.dma_start(out=out_d[nb * 128:(nb + 1) * 128, :],
                                  in_=ost[:])

        # ---------------- S8: loss partial sums ----------------------------
        with tc.tile_pool(name="lsp", bufs=1) as lsp:
            sq3 = lsp.tile([128, NB * 8], F32, tag="sq3")
            nc.scalar.sqrt(sq3[:], nrm2[:].rearrange("p a h -> p (a h)"))
            rsn32 = lsp.tile([128, NB * 8], F32, tag="rsn32")
            nc.vector.reciprocal(rsn32[:], sq3[:])
            rsn16 = lsp.tile([128, NB * 8], F16, tag="rsn16")
            nc.vector.tensor_copy(rsn16[:], rsn32[:])
            rsnb = rsn16[:].rearrange("p (a h) -> p a h", h=8).unsqueeze(
                3).broadcast_to([128, NB, 8, 16])
            val2 = lsp.tile([128, 2], F32, tag="val2")
            for bank in range(2):
                s4 = ssb[:, :, bank * 128:(bank + 1) * 128].rearrange(
                    "p a (h m) -> p a h m", m=16)
                r4 = rsb[:, bank].rearrange("p a (h m) -> p a h m", m=16)
                pp = lsp.tile([128, NB, 128], F16, tag="pp")
                pp4 = pp[:].rearrange("p a (h m) -> p a h m", m=16)
                nc.vector.tensor_tensor(pp4, s4, r4, op=OP.mult)
                posn = lsp.tile([128, NB * 8], F32, tag=f"posn_{bank}")
                nc.vector.tensor_reduce(posn[:], pp4, axis=AX.X, op=OP.add)
                sc = lsp.tile([128, NB, 128], F16, tag="sc")
                sc4 = sc[:].rearrange("p a (h m) -> p a h m", m=16)
                nc.vector.tensor_tensor(sc4, s4, rsnb, op=OP.mult)
                ex = lsp.tile([128, NB, 128], F16, tag="ex")
                nc.scalar.activation(ex[:], sc[:], AF.Exp, scale=1.0 / TAU)
                S = lsp.tile([128, NB * 8], F32, tag=f"S_{bank}")
                nc.vector.tensor_reduce(
                    S[:], ex[:].rearrange("p a (h m) -> p a h m", m=16),
                    axis=AX.X, op=OP.add)
                lse = lsp.tile([128, NB * 8], F32, tag=f"lse_{bank}")
                nc.scalar.activation(lse[:], S[:], AF.Ln)
                tmp = lsp.tile([128, NB * 8], F32, tag="tmp")
                nc.vector.tensor_tensor(tmp[:], posn[:], rsn32[:], op=OP.mult)
                valc = lsp.tile([128, NB * 8], F32, tag="valc")
                nc.vector.scalar_tensor_tensor(
                    valc[:], tmp[:], 1.0 / TAU, lse[:],
                    op0=OP.mult, op1=OP.subtract)
                nc.vector.tensor_reduce(val2[:, bank:bank + 1], valc[:],
                                        axis=AX.X, op=OP.add)
            with tc.tile_pool(name="ps_l", bufs=1, space="PSUM") as plp:
                pl = plp.tile([1, 2], F32)
                nc.tensor.matmul(pl[:], ones32[:, 0:1], val2[:],
                                 start=True, stop=True)
                lst = small.tile([1, 2], F32)
                nc.scalar.copy(lst[:], pl[:])
                nc.sync.dma_start(out=loss_d[:], in_=lst[:])

    nc.compile()
    return nc


def _l2n(a, eps):
    n = np.linalg.norm(a, axis=-1, keepdims=True)
    return a / np.maximum(n, eps)


def _blockdiag(blocks):
    """blocks: [H, r, c] -> [H*r, H*c] block diagonal."""
    h, r, c = blocks.shape
    out = np.zeros((h * r, h * c), np.float32)
    for i in range(h):
        out[i * r:(i + 1) * r, i * c:(i + 1) * c] = blocks[i]
    return out


def _to_chunked(a, f16=True):
    """[512, K] -> [128, 4, K] with rows c = cb*128 + p."""
    a = a.astype(np.float16) if f16 else a
    return np.ascontiguousarray(a.reshape(4, 128, -1).transpose(1, 0, 2))


def prep_host(inputs, NB=32):
    x = np.asarray(inputs["x"], np.float32)
    W_qkv = np.asarray(inputs["W_qkv"], np.float32)
    W_proj = np.asarray(inputs["W_proj"], np.float32)
    b_proj = np.asarray(inputs["b_proj"], np.float32)
    W_rspec = np.asarray(inputs["W_rspec"], np.float32)
    b_rspec = np.asarray(inputs["b_rspec"], np.float32)
    W_rspat = np.asarray(inputs["W_rspat"], np.float32)
    b_rspat = np.asarray(inputs["b_rspat"], np.float32)
    K_spec = np.asarray(inputs["K_spec"], np.float32)
    K_spat = np.asarray(inputs["K_spat"], np.float32)
    W_tok = np.asarray(inputs["W_tok"], np.float32)
    b_tok = np.asarray(inputs["b_tok"], np.float32)
    W_prot = np.asarray(inputs["W_prot"], np.float32)
    b_prot = np.asarray(inputs["b_prot"], np.float32)

    Wq = W_qkv[:, 0:C]
    Wv = W_qkv[:, 2 * C:3 * C]

    pn_spec = _l2n(K_spec @ W_prot + b_prot, 1e-12)   # [H, M, D]
    pn_spat = _l2n(K_spat @ W_prot + b_prot, 1e-12)

    A_spec = np.stack([W_tok @ pn_spec[h].T for h in range(H)])  # [H, D, M]
    A_spat = np.stack([W_tok @ pn_spat[h].T for h in range(H)])
    Ws_spec = Wv @ _blockdiag(A_spec)
    Ws_spat = Wv @ _blockdiag(A_spat)
    Wt = Wv @ _blockdiag(np.broadcast_to(W_tok, (H, D, D)))

    Wbig = np.concatenate([Wv, Wt, W_rspec, W_rspat, Ws_spec, Ws_spat], axis=1)

    bcat = np.concatenate([_blockdiag(K_spec), _blockdiag(K_spat)],
                          axis=1).astype(np.float16)              # [128, 1024]

    c_spec = (pn_spec @ b_tok).reshape(-1)   # [H*M] bias of s_raw
    c_spat = (pn_spat @ b_tok).reshape(-1)
    brow_big = np.concatenate([
        np.zeros(C, np.float32), np.tile(b_tok, H),
        b_rspec, b_rspat, c_spec, c_spat])
    brow = np.concatenate([brow_big, b_proj])[None, :].astype(np.float16)
    with_big_bias = bool(np.any(brow_big))
    with_proj_bias = bool(np.any(b_proj))

    weights = {
        "wq": _to_chunked(Wq),
        "wbig": _to_chunked(Wbig),
        "bcat": bcat,
        "wproj": _to_chunked(W_proj),
        "brow": brow,
    }

    Np = NB * 128
    qseg = min(512, Np)
    NSEG = Np // qseg
    in_maps = []
    for b in range(x.shape[0]):
        xT = np.ascontiguousarray(
            x[b, :Np].astype(np.float16).reshape(NSEG, qseg, 4, 128)
            .transpose(3, 0, 2, 1))
        m = dict(weights)
        m["xT"] = xT
        in_maps.append(m)

    def divloss(P):
        p = P / np.maximum(np.linalg.norm(P, axis=-1, keepdims=True), 1e-6)
        sim = np.einsum("hmd,hnd->hmn", p, p)
        mask = 1.0 - np.eye(P.shape[1], dtype=P.dtype)[None]
        return np.sum(sim * mask) / (P.shape[0] * P.shape[1] * (P.shape[1] - 1) + 1e-6)

    diversity = np.float32(DIV_REG * (divloss(K_spec) + divloss(K_spat)))
    return in_maps, with_big_bias, with_proj_bias, diversity


_NC_CACHE = {}
LAST_RESULTS = None


def kernel(**inputs):
    global LAST_RESULTS
    in_maps, bb, pb, diversity = prep_host(inputs)
    key = (32, bb, pb)
    if key not in _NC_CACHE:
        _NC_CACHE[key] = build_core_kernel(32, bb, pb)
    nc = _NC_CACHE[key]
    res = run_bass_kernel_spmd(nc, in_maps, core_ids=list(range(NCORES)))
    LAST_RESULTS = res
    out = np.stack([res.results[c]["out"] for c in range(NCORES)])
    ls = np.stack([res.results[c]["lossp"][0] for c in range(NCORES)])
    tot = ls.sum(axis=0)
    denom = B * H * N
    contrastive = np.float32(0.5 * (-(tot[0] / denom) - (tot[1] / denom)))
    return out, contrastive, diversity
